# revision 1
# baseline (speedup 1.0000x reference)
"""MoE decoder Trainium2 kernel (nn_MoEDecoder_67654324846797) — v2 bf16.

Strategy
--------
Data-parallel: the token dim (N=65536) is sharded across 8 NeuronCores
(8192 tokens each); all weights are replicated. No collectives.

Per-core kernel (feature-major, weight-stationary, bf16 matmuls):
  - All matmul operands are bf16 (host casts inputs; fp32 PSUM accumulation;
    ~5e-3 end-to-end rel err). FWL weight loads run 2x faster than f32r,
    DVE/ACT elementwise ops on SBUF operands hit 2x/4x perf modes, and all
    DMA traffic is halved vs f32r.
  - x tiles transpose to feature-major on the PE into kt-pair bf16 PSUM
    tiles ([128,2,512], one bank) so the 2-buffer rotation never stalls the
    PE on a same-tile DVE evacuation; output transposes back the same way.
    (DMA xbar transpose was tried: InstDmaTransposeAnt races
    nondeterministically at 16-row granularity on this stack.)
  - Softmax is unnormalized on the expert path: the broadcast carries
    exp(l_e) and the 1/Z division folds into the token-major output
    evacuation as a per-partition tensor_scalar_mul (Z reshaped
    [1,512]->[128,4] via a DRAM bounce so the DVE reciprocal runs 128-wide,
    178ns instead of 3.3us; also kills one broadcast chain + probT mul).
  - PSUM evacuations balanced across DVE/ACT (L1 alternates per expert,
    L2 on ACT); gate multiply on DVE in 2x bf16 mode. The 1/Z reciprocal
    is issued in phase B next to its consumer: in phase A it sat in the
    strict-FIFO DVE queue waiting on the gpsimd Z-bounce and blocked
    phase B's PSUM evacuations behind it (2-5us PE stalls per tile pair).
  - Phase A (x load/transpose, gating, exp/Z bounce, broadcast) runs 3
    tiles ahead of phase B (experts); x prefetch precedes the weight
    stream so tile 0 starts immediately.
  - For even time_reps the timing loop unrolls two logical repeats per
    For_i body, software-pipelined through the junction: the pipeline
    drain at the hardware-loop boundary (the broadcast-chain latency for
    tile 0, ~12-18us) is paid once per two repeats instead of every one.
    (A rotation fully across the For_i boundary deadlocks the tile
    scheduler - bodies must be self-contained.)
"""

import numpy as np

import concourse.bass as bass
import concourse.tile as tile
from concourse import bacc, mybir
from concourse.masks import make_identity

F32 = mybir.dt.float32
BF16 = mybir.dt.bfloat16

N_TOKENS = 65536
N_CORES = 8
TOK_PER_CORE = N_TOKENS // N_CORES  # 8192
TILE = 512  # tokens per tile
N_TILES = TOK_PER_CORE // TILE  # 16
IN_CH = 512
HID = 256
OUT_CH = 256
E = 8

RELU = mybir.ActivationFunctionType.Relu
EXP = mybir.ActivationFunctionType.Exp
COPY = mybir.ActivationFunctionType.Copy

# Empirically determined xbar dst row enumeration for 3D outputs:
# "p_outer": logical row r of the transposed result lands at (partition
# r // inner, slot r % inner); "s_outer": at (partition r % 128, slot r // 128).
XBAR_P_OUTER = False


def build_kernel(time_reps: int = 1) -> bass.Bass:
    nc = bacc.Bacc("TRN2", target_bir_lowering=False, debug=False,
                   num_devices=N_CORES)

    x = nc.dram_tensor("x", [TOK_PER_CORE, IN_CH], BF16, kind="ExternalInput").ap()
    eW1 = nc.dram_tensor("eW1", [E, IN_CH, HID], BF16, kind="ExternalInput").ap()
    eb1 = nc.dram_tensor("eb1", [E, HID], F32, kind="ExternalInput").ap()
    eW2 = nc.dram_tensor("eW2", [E, HID, HID], BF16, kind="ExternalInput").ap()
    eb2 = nc.dram_tensor("eb2", [E, HID], F32, kind="ExternalInput").ap()
    eW3 = nc.dram_tensor("eW3", [E, HID, OUT_CH], BF16, kind="ExternalInput").ap()
    eb3 = nc.dram_tensor("eb3", [E, OUT_CH], BF16, kind="ExternalInput").ap()
    gW1 = nc.dram_tensor("gW1", [IN_CH, HID], BF16, kind="ExternalInput").ap()
    gb1 = nc.dram_tensor("gb1", [HID], F32, kind="ExternalInput").ap()
    gW2 = nc.dram_tensor("gW2", [HID, HID], BF16, kind="ExternalInput").ap()
    gb2 = nc.dram_tensor("gb2", [HID], F32, kind="ExternalInput").ap()
    gW3 = nc.dram_tensor("gW3", [HID, E], BF16, kind="ExternalInput").ap()
    gb3 = nc.dram_tensor("gb3", [E], F32, kind="ExternalInput").ap()
    out = nc.dram_tensor("out", [TOK_PER_CORE, OUT_CH], BF16,
                         kind="ExternalOutput").ap()

    with tile.TileContext(nc) as tc:
        _body(nc, tc, x, eW1, eb1, eW2, eb2, eW3, eb3,
              gW1, gb1, gW2, gb2, gW3, gb3, out, time_reps)
    nc.compile()
    return nc


def _body(nc, tc, x, eW1, eb1, eW2, eb2, eW3, eb3,
          gW1, gb1, gW2, gb2, gW3, gb3, out, time_reps):
    from contextlib import ExitStack

    ctx = ExitStack()
    with ctx:
        wpool = ctx.enter_context(tc.tile_pool(name="wpool", bufs=1))
        act_pool = ctx.enter_context(tc.tile_pool(name="act", bufs=2))
        small_pool = ctx.enter_context(tc.tile_pool(name="small", bufs=3))
        io_pool = ctx.enter_context(tc.tile_pool(name="io", bufs=4))
        ps_mlp = ctx.enter_context(tc.tile_pool(name="ps_mlp", bufs=4, space="PSUM"))
        ps_out = ctx.enter_context(tc.tile_pool(name="ps_out", bufs=1, space="PSUM"))
        ps_tr = ctx.enter_context(tc.tile_pool(name="ps_tr", bufs=2, space="PSUM"))
        dram_pool = ctx.enter_context(tc.tile_pool(name="dram", bufs=4, space="DRAM"))

        # ---- prefetch x for tiles 0-2 ahead of the weight stream ----
        x_r = x.rearrange("(t s p) f -> t p s f", p=128, s=4)
        x_nat_t = {}

        def load_x(t, key=None):
            x_nat = io_pool.tile([128, 4, IN_CH], BF16, name="x_nat")
            nc.sync.dma_start(x_nat, x_r[t])
            x_nat_t[t if key is None else key] = x_nat

        if time_reps == 1:
            load_x(0)
            load_x(1)
            load_x(2)

        # ---- weight preload (feature-major, stationary layouts) ----
        g1w = wpool.tile([128, 4, HID], BF16, name="g1w")
        nc.sync.dma_start(g1w, gW1.rearrange("(kt kp) m -> kp kt m", kp=128))
        g2w = wpool.tile([128, 2, HID], BF16, name="g2w")
        nc.sync.dma_start(g2w, gW2.rearrange("(kt kp) m -> kp kt m", kp=128))
        g3w = wpool.tile([128, 2, E], BF16, name="g3w")
        nc.sync.dma_start(g3w, gW3.rearrange("(kt kp) m -> kp kt m", kp=128))
        g1b = wpool.tile([128, 2], F32, name="g1b")
        nc.sync.dma_start(g1b, gb1.rearrange("(mt mp) -> mp mt", mp=128))
        g2b = wpool.tile([128, 2], F32, name="g2b")
        nc.sync.dma_start(g2b, gb2.rearrange("(mt mp) -> mp mt", mp=128))
        g3b = wpool.tile([E, 1], F32, name="g3b")
        nc.sync.dma_start(g3b, gb3.rearrange("(e one) -> e one", one=1))
        b1e = wpool.tile([128, E, 2], F32, name="b1e")
        nc.sync.dma_start(b1e, eb1.rearrange("e (mt mp) -> mp e mt", mp=128))
        b2e = wpool.tile([128, E, 2], F32, name="b2e")
        nc.sync.dma_start(b2e, eb2.rearrange("e (mt mp) -> mp e mt", mp=128))
        b3e = wpool.tile([E, OUT_CH], BF16, name="b3e")  # lhsT for bias matmul
        nc.sync.dma_start(b3e, eb3)
        ones8f = wpool.tile([E, 1], F32, name="ones8f")
        nc.vector.memset(ones8f, 1.0)
        ones8 = wpool.tile([E, 1], BF16, name="ones8")
        nc.vector.tensor_copy(ones8, ones8f)
        identf = wpool.tile([128, 128], F32, name="identf")
        make_identity(nc, identf)
        identb = wpool.tile([128, 128], BF16, name="identb")
        nc.vector.tensor_copy(identb, identf)

        w1e = wpool.tile([128, E, 4, HID], BF16, name="w1e")
        w2e = wpool.tile([128, E, 2, HID], BF16, name="w2e")
        w3e = wpool.tile([128, E, 2, OUT_CH], BF16, name="w3e")
        eW1r = eW1.rearrange("e (kt kp) m -> e kp kt m", kp=128)
        eW2r = eW2.rearrange("e (kt kp) m -> e kp kt m", kp=128)
        eW3r = eW3.rearrange("e (kt kp) m -> e kp kt m", kp=128)
        rings = [nc.sync, nc.scalar, nc.gpsimd]
        for e in range(E):
            ring = rings[e % 3]
            ring.dma_start(w1e[:, e], eW1r[e])
            ring.dma_start(w2e[:, e], eW2r[e])
            ring.dma_start(w3e[:, e], eW3r[e])

        out_r = out.rearrange("(t s p) o -> t p s o", p=128, s=4)

        # Pipelined 2-phase structure: phase A (xbar x load, gating MLP,
        # exp/Z bounce + broadcast DMA chain) runs 2 tiles ahead of phase B
        # (experts) so the DRAM-bounce latency hides behind B's PE work.
        xT_t, wbc_t, expT_t, rP_t = {}, {}, {}, {}

        def phase_a(t, key=None):
            key = t if key is None else key
            if key not in x_nat_t:
                load_x(t, key)
            x_nat = x_nat_t.pop(key)
            xT = act_pool.tile([128, 4, TILE], BF16, name="xT", bufs=4)
            for ktp in range(2):
                p_tr = ps_tr.tile([128, 2, TILE], BF16, name="p_tr", tag="ptr")
                for k2 in range(2):
                    kt = ktp * 2 + k2
                    for sj in range(4):
                        nc.tensor.transpose(
                            p_tr[:, k2, sj * 128:(sj + 1) * 128],
                            x_nat[:, sj, kt * 128:(kt + 1) * 128], identb)
                nc.vector.tensor_copy(xT[:, ktp * 2:(ktp + 1) * 2, :], p_tr)

            g1T = act_pool.tile([128, 2, TILE], BF16, name="g1T", bufs=2)
            for mt in range(2):
                p_g = ps_mlp.tile([128, TILE], F32, name="p_g", tag="pmlp")
                for kt in range(4):
                    nc.tensor.matmul(p_g, g1w[:, kt, mt * 128:(mt + 1) * 128],
                                     xT[:, kt, :], start=(kt == 0), stop=(kt == 3))
                nc.scalar.activation(g1T[:, mt, :], p_g, RELU, bias=g1b[:, mt:mt + 1])
            g2T = act_pool.tile([128, 2, TILE], BF16, name="g2T", bufs=2)
            for mt in range(2):
                p_g2 = ps_mlp.tile([128, TILE], F32, name="p_g2", tag="pmlp")
                for kt in range(2):
                    nc.tensor.matmul(p_g2, g2w[:, kt, mt * 128:(mt + 1) * 128],
                                     g1T[:, kt, :], start=(kt == 0), stop=(kt == 1))
                nc.scalar.activation(g2T[:, mt, :], p_g2, RELU, bias=g2b[:, mt:mt + 1])
            p_l = ps_tr.tile([E, TILE], F32, name="p_l", tag="ptr")
            for kt in range(2):
                nc.tensor.matmul(p_l, g3w[:, kt, :], g2T[:, kt, :],
                                 start=(kt == 0), stop=(kt == 1))
            expT = small_pool.tile([E, TILE], BF16, name="expT", bufs=4)
            nc.scalar.activation(expT, p_l, EXP, bias=g3b)

            # Z = sum_e exp_e (unnormalized); 1/Z is applied token-major at
            # the output. Z bounces through DRAM to reshape [1,512]->[128,4].
            p_z = ps_tr.tile([1, TILE], F32, name="p_z", tag="ptr")
            nc.tensor.matmul(p_z, ones8, expT, start=True, stop=True)
            z_sb = small_pool.tile([1, TILE], F32, name="z_sb")
            nc.scalar.activation(z_sb, p_z, COPY)
            z_dram = dram_pool.tile([1, TILE], F32, name="z_dram")
            nc.gpsimd.dma_start(z_dram, z_sb)
            zP = small_pool.tile([128, 4], F32, name="zP", bufs=4)
            nc.gpsimd.dma_start(zP, z_dram[0].rearrange("(s p) -> p s", p=128))

            exp_dram = dram_pool.tile([E, TILE], BF16, name="exp_dram")
            nc.gpsimd.dma_start(exp_dram, expT)
            w_bc = act_pool.tile([128, E, TILE], BF16, name="w_bc", bufs=4)
            for e in range(E):
                nc.gpsimd.dma_start(
                    w_bc[:, e], exp_dram[e, :].partition_broadcast(128))
            xT_t[key], wbc_t[key], expT_t[key], rP_t[key] = xT, w_bc, expT, zP

        def phase_b(t, key=None):
            key = t if key is None else key
            xT, w_bc, expT, zP = (xT_t.pop(key), wbc_t.pop(key),
                                  expT_t.pop(key), rP_t.pop(key))
            p_o = [ps_out.tile([128, TILE], F32, name=f"p_o{mt}", tag=f"po{mt}")
                   for mt in range(2)]
            for e in range(E):
                h1T = act_pool.tile([128, 2, TILE], BF16, name="h1T", bufs=3)
                for mt in range(2):
                    p_h = ps_mlp.tile([128, TILE], F32, name="p_h", tag="pmlp")
                    for kt in range(4):
                        nc.tensor.matmul(p_h, w1e[:, e, kt, mt * 128:(mt + 1) * 128],
                                         xT[:, kt, :], start=(kt == 0), stop=(kt == 3))
                    # balance PSUM evacuations: alternate DVE / ACT
                    if e % 2 == 0:
                        nc.vector.tensor_scalar(
                            h1T[:, mt, :], p_h, b1e[:, e, mt:mt + 1], 0.0,
                            mybir.AluOpType.add, mybir.AluOpType.max)
                    else:
                        nc.scalar.activation(h1T[:, mt, :], p_h, RELU,
                                             bias=b1e[:, e, mt:mt + 1])
                h2s = act_pool.tile([128, 2, TILE], BF16, name="h2s")
                for mt in range(2):
                    p_h2 = ps_mlp.tile([128, TILE], F32, name="p_h2", tag="pmlp")
                    for kt in range(2):
                        nc.tensor.matmul(p_h2, w2e[:, e, kt, mt * 128:(mt + 1) * 128],
                                         h1T[:, kt, :], start=(kt == 0), stop=(kt == 1))
                    h2T = act_pool.tile([128, TILE], BF16, name="h2T", bufs=3)
                    nc.scalar.activation(h2T, p_h2, RELU, bias=b2e[:, e, mt:mt + 1])
                    nc.vector.tensor_mul(h2s[:, mt, :], h2T, w_bc[:, e])
                for mt in range(2):
                    for kt in range(2):
                        nc.tensor.matmul(p_o[mt], w3e[:, e, kt, mt * 128:(mt + 1) * 128],
                                         h2s[:, kt, :],
                                         start=(e == 0 and kt == 0), stop=False,
                                         skip_group_check=True)

            # gated bias: p_o[mt] += eb3.T[mt-slice] @ expT (unnormalized)
            for mt in range(2):
                nc.tensor.matmul(p_o[mt], b3e[:, mt * 128:(mt + 1) * 128], expT,
                                 start=False, stop=True, skip_group_check=True)

            rP = small_pool.tile([128, 4], F32, name="rP", bufs=2)
            nc.vector.reciprocal(rP, zP)
            outT = act_pool.tile([128, 2, TILE], BF16, name="outT")
            nc.vector.tensor_copy(outT[:, 0, :], p_o[0])
            nc.vector.tensor_copy(outT[:, 1, :], p_o[1])
            # transpose back token-major on the PE; the 1/Z gate-normalization
            # folds into the PSUM evacuation as a per-partition scalar mul
            out_tok = io_pool.tile([128, 4, OUT_CH], BF16, name="out_tok")
            for s_ in range(4):
                p_ot = ps_out.tile([128, OUT_CH], BF16, name="p_ot", tag=f"po{s_ % 2}")
                for mt in range(2):
                    nc.tensor.transpose(
                        p_ot[:, mt * 128:(mt + 1) * 128],
                        outT[:, mt, s_ * 128:(s_ + 1) * 128], identb)
                nc.vector.tensor_scalar_mul(out_tok[:, s_, :], p_ot,
                                            rP[:, s_:s_ + 1])
            nc.sync.dma_start(out_r[t], out_tok)

        def main_loop():
            if time_reps > 1:
                load_x(0)
                load_x(1)
            phase_a(0)
            phase_a(1)
            phase_a(2)
            for t in range(N_TILES):
                if t + 3 < N_TILES:
                    phase_a(t + 3)
                phase_b(t)

        def main_loop_n(n):
            U = n * N_TILES
            phase_a(0, 0)
            phase_a(1, 1)
            phase_a(2, 2)
            for u in range(U):
                if u + 3 < U:
                    phase_a((u + 3) % N_TILES, u + 3)
                phase_b(u % N_TILES, u)

        if time_reps > 1 and time_reps % 8 == 0:
            with tc.For_i(0, time_reps // 8, 1):
                main_loop_n(8)
        elif time_reps > 1 and time_reps % 4 == 0:
            with tc.For_i(0, time_reps // 4, 1):
                main_loop_n(4)
        elif time_reps > 1 and time_reps % 2 == 0:
            with tc.For_i(0, time_reps // 2, 1):
                main_loop_n(2)
        elif time_reps > 1:
            with tc.For_i(0, time_reps, 1):
                main_loop()
        else:
            main_loop()


# ---------------------------------------------------------------------------
# PJRT runner (self-contained; mirrors concourse.bass2jax.run_bass_via_pjrt
# but keeps the jitted callable + device inputs for repeat timing)
# ---------------------------------------------------------------------------
class BassRunner:
    def __init__(self, nc: bass.Bass, n_cores: int = 8):
        import jax
        from jax.sharding import Mesh, PartitionSpec
        from jax.experimental.shard_map import shard_map
        from concourse.bass2jax import (
            _bass_exec_p, install_neuronx_cc_hook, partition_id_tensor,
        )

        install_neuronx_cc_hook()
        self.jax = jax
        self.nc = nc
        self.n_cores = n_cores
        partition_name = (
            nc.partition_id_tensor.name if nc.partition_id_tensor else None
        )

        in_names, out_names, out_avals, zero_shapes = [], [], [], []
        for alloc in nc.m.functions[0].allocations:
            if not isinstance(alloc, mybir.MemoryLocationSet):
                continue
            name = alloc.memorylocations[0].name
            if alloc.kind == "ExternalInput":
                if name != partition_name:
                    in_names.append(name)
            elif alloc.kind == "ExternalOutput":
                shape = tuple(alloc.tensor_shape)
                np_dt = mybir.dt.np(alloc.dtype)
                out_names.append(name)
                out_avals.append(jax.core.ShapedArray(shape, np_dt))
                zero_shapes.append((shape, np_dt))

        self.in_names, self.out_names = in_names, out_names
        self.out_avals, self.zero_shapes = out_avals, zero_shapes
        n_params, n_outs = len(in_names), len(out_names)
        bind_in_names = in_names + out_names
        if partition_name is not None:
            bind_in_names.append(partition_name)

        def _b(*args):
            operands = list(args)
            if partition_name is not None:
                operands.append(partition_id_tensor())
            return tuple(_bass_exec_p.bind(
                *operands,
                out_avals=tuple(out_avals),
                in_names=tuple(bind_in_names),
                out_names=tuple(out_names),
                lowering_input_output_aliases=(),
                sim_require_finite=True,
                sim_require_nnan=True,
                nc=nc,
            ))

        devices = jax.devices()[:n_cores]
        assert len(devices) == n_cores
        self.mesh = Mesh(np.asarray(devices), ("core",))
        self.pspec = PartitionSpec("core")
        in_specs = (self.pspec,) * (n_params + n_outs)
        out_specs = (self.pspec,) * n_outs
        self.sharded = jax.jit(
            shard_map(_b, mesh=self.mesh, in_specs=in_specs,
                      out_specs=out_specs, check_rep=False),
            keep_unused=True,
        )
        self._dev_in = None

    def put_inputs(self, in_maps):
        import jax
        concat = [
            np.concatenate([in_maps[c][n] for c in range(self.n_cores)], axis=0)
            for n in self.in_names
        ]
        zeros = [
            np.zeros((self.n_cores * s[0], *s[1:]), d) for s, d in self.zero_shapes
        ]
        sh = jax.sharding.NamedSharding(self.mesh, self.pspec)
        self._dev_in = [jax.device_put(a, sh) for a in concat + zeros]
        jax.block_until_ready(self._dev_in)

    def run(self):
        out = self.sharded(*self._dev_in)
        self.jax.block_until_ready(out)
        return out

    def results(self, out):
        res = []
        for c in range(self.n_cores):
            d = {}
            for i, name in enumerate(self.out_names):
                arr = np.asarray(out[i]).reshape(
                    self.n_cores, *self.out_avals[i].shape)
                d[name] = arr[c]
            res.append(d)
        return res

    def time_runs(self, iters=10, warmup=2):
        import time
        for _ in range(warmup):
            self.run()
        times = []
        for _ in range(iters):
            t0 = time.perf_counter()
            self.run()
            times.append(time.perf_counter() - t0)
        return min(times), sum(times) / len(times)


_cached = {}


def _get_runner(time_reps: int = 1) -> BassRunner:
    if time_reps not in _cached:
        nc = build_kernel(time_reps)
        _cached[time_reps] = BassRunner(nc, N_CORES)
    return _cached[time_reps]


def _in_maps(inputs: dict) -> list:
    import ml_dtypes
    bf = ml_dtypes.bfloat16
    f32_keys = {"eb1", "eb2", "gb1", "gb2", "gb3"}
    shared = {}
    for k, v in inputs.items():
        if k == "x":
            continue
        a = np.ascontiguousarray(np.asarray(v, dtype=np.float32))
        shared[k] = a if k in f32_keys else np.ascontiguousarray(a.astype(bf))
    x_full = np.ascontiguousarray(
        np.asarray(inputs["x"], dtype=np.float32).astype(bf))
    maps = []
    for c in range(N_CORES):
        m = dict(shared)
        m["x"] = x_full[c * TOK_PER_CORE:(c + 1) * TOK_PER_CORE]
        maps.append(m)
    return maps


def kernel(**inputs) -> np.ndarray:
    runner = _get_runner(1)
    runner.put_inputs(_in_maps(inputs))
    res = runner.results(runner.run())
    full = np.concatenate([r["out"] for r in res], axis=0)
    return full.astype(np.float32)

